# revision 15
# baseline (speedup 1.0000x reference)
"""Multi-head rotary attention block on 8 Trainium2 NeuronCores.

Sharding: tensor-parallel over heads (16 heads / 8 cores = 2 heads per core,
both batches on every core); one 8-way AllToAll redistributes the attention
output from head-sharded to sequence-sharded for the output projection, so
each core finishes layernorm on its own [512, 1024] output slice.

Per-core dataflow (feature-major "T" layouts are [channels, seq]):
  qT/kT = w_q^T x^T (+bias via K=1 matmul) with RoPE applied as
          raw*cosM + (SpermT^T raw)*sinM (rotation permutation as a matmul)
  vT    = w_v^T x^T, PE-transposed to natural v [seq, 128]
  per head: sT[j,i] = kT^T qT (transposed scores), pT = exp(sT/32) on ScalarE
  (no max subtraction: scores are O(0.5) under this operator's input law),
  softmax denominators D via ones-matmul over pT, 1/D = exp(-ln(D)) on ScalarE,
  xvT = (v^T pT) * (1/D); AllToAll; z = xv_gathered + x residual;
  y = z^T w_out + b_out; layernorm via bn_stats/bn_aggr + exp(-0.5 ln(var+eps)).

All matmuls run as float32r (full-rate fp32 PE mode; 4-byte data, producers
write into tiles declared float32r as the BIR verifier requires).
"""
import numpy as np

import concourse.bass as bass
import concourse.bacc as bacc
import concourse.tile as tile
import concourse.mybir as mybir
from concourse import bass_utils

F32 = mybir.dt.float32
F32R = mybir.dt.float32r
AF = mybir.ActivationFunctionType
ALU = mybir.AluOpType

NCORES = 8
B, S, D = 2, 2048, 1024
HEADS, HD = 16, 64
SCALE = 1.0 / float(np.sqrt(D))  # reference scales by full D, not head_dim
IT = 512          # i-tile width for attention
N_IT = S // IT    # 4
JC = 128          # j-chunk
N_JC = S // JC    # 16
N_EC = D // 128   # 8 e-chunks


def _rope_consts():
    rot = HD // 2
    inv_freq = 1.0 / (10000.0 ** (np.arange(0, rot, 2, dtype=np.float64) / rot))
    ang = np.arange(S, dtype=np.float64)[:, None] * inv_freq[None, :]
    ang = np.repeat(ang, 2, axis=-1)  # [S, 32]
    cos, sin = np.cos(ang), np.sin(ang)
    cosM = np.ones((128, S), dtype=np.float32)
    sinM = np.zeros((128, S), dtype=np.float32)
    for base in (0, 64):
        cosM[base : base + 32, :] = cos.T.astype(np.float32)
        sinM[base : base + 32, :] = sin.T.astype(np.float32)
    Sp = np.zeros((128, 128), dtype=np.float32)
    for base in (0, 64):
        for m in range(32):
            r0 = base + m
            if m % 2 == 0:
                Sp[r0, r0 + 1] = -1.0
            else:
                Sp[r0, r0 - 1] = 1.0
    SpermT = np.ascontiguousarray(Sp.T)
    return cosM, sinM, SpermT


def _build(sim=False):
    nc = bacc.Bacc("TRN2", target_bir_lowering=False, debug=False, num_devices=NCORES)

    xT_d = [nc.dram_tensor(f"xT{b}", [D, S], F32R, kind="ExternalInput") for b in range(B)]
    wq_d = nc.dram_tensor("wq", [D, 128], F32R, kind="ExternalInput")
    wk_d = nc.dram_tensor("wk", [D, 128], F32R, kind="ExternalInput")
    wv_d = nc.dram_tensor("wv", [D, 128], F32R, kind="ExternalInput")
    bq_d = nc.dram_tensor("bq", [1, 128], F32R, kind="ExternalInput")
    bk_d = nc.dram_tensor("bk", [1, 128], F32R, kind="ExternalInput")
    bv_d = nc.dram_tensor("bv", [1, 128], F32R, kind="ExternalInput")
    wout_d = nc.dram_tensor("wout", [D, D], F32R, kind="ExternalInput")
    bout_d = nc.dram_tensor("bout", [1, D], F32R, kind="ExternalInput")
    gamma_d = nc.dram_tensor("gamma", [1, D], F32R, kind="ExternalInput")
    beta_d = nc.dram_tensor("beta", [1, D], F32R, kind="ExternalInput")
    cosM_d = nc.dram_tensor("cosM", [128, S], F32, kind="ExternalInput")
    sinM_d = nc.dram_tensor("sinM", [128, S], F32, kind="ExternalInput")
    spt_d = nc.dram_tensor("SpermT", [128, 128], F32R, kind="ExternalInput")
    ident_d = nc.dram_tensor("ident", [128, 128], F32, kind="ExternalInput")
    xres_d = nc.dram_tensor("xres", [D, 512], F32, kind="ExternalInput")
    y_d = nc.dram_tensor("y_out", [512, D], F32, kind="ExternalOutput")

    with tile.TileContext(nc) as tc:
        with (
            tc.tile_pool(name="persist", bufs=1) as pp,
            tc.tile_pool(name="dram", bufs=1, space="DRAM") as dram,
        ):
            cosM = pp.tile([128, S], F32, name="cosM_sb")
            sinM = pp.tile([128, S], F32, name="sinM_sb")
            spt = pp.tile([128, 128], F32R, name="spt_sb")
            ident = pp.tile([128, 128], F32, name="ident_sb")
            nc.sync.dma_start(cosM[:], cosM_d.ap())
            nc.sync.dma_start(sinM[:], sinM_d.ap())
            nc.sync.dma_start(spt[:], spt_d.ap())
            nc.sync.dma_start(ident[:], ident_d.ap())

            wq = pp.tile([128, N_EC, 128], F32R, name="wq_sb")  # [p, ec, m]
            wk = pp.tile([128, N_EC, 128], F32R, name="wk_sb")
            wv = pp.tile([128, N_EC, 128], F32R, name="wv_sb")
            for w_sb, w_dd in ((wq, wq_d), (wk, wk_d), (wv, wv_d)):
                nc.sync.dma_start(
                    w_sb[:], w_dd.ap().rearrange("(c p) m -> p c m", p=128))
            bq = pp.tile([1, 128], F32R, name="bq_sb")
            bk = pp.tile([1, 128], F32R, name="bk_sb")
            bv = pp.tile([1, 128], F32R, name="bv_sb")
            nc.sync.dma_start(bq[:], bq_d.ap())
            nc.sync.dma_start(bk[:], bk_d.ap())
            nc.sync.dma_start(bv[:], bv_d.ap())
            ones_row = pp.tile([1, IT], F32R, name="ones_row")
            nc.vector.memset(ones_row[:].bitcast(F32), 1.0)
            ones128 = pp.tile([128, 128], F32R, name="ones128")
            nc.vector.memset(ones128[:].bitcast(F32), 1.0)
            ones_col = pp.tile([1, 128], F32R, name="ones_col")
            nc.vector.memset(ones_col[:].bitcast(F32), 1.0)

            xvT = [pp.tile([128, S], F32, name=f"xvT_{b}") for b in range(B)]

            with (
                tc.tile_pool(name="psp", bufs=1, space="PSUM") as psp,
                tc.tile_pool(name="psa", bufs=1, space="PSUM") as psa,
                tc.tile_pool(name="ptmp", bufs=2) as ptmp,
                tc.tile_pool(name="ptp", bufs=4) as ptp,
                tc.tile_pool(name="rp", bufs=2) as rp,
            ):
             for b in range(B):
              with tc.tile_pool(name=f"qkv{b}", bufs=1) as qkvp:
                qTb = qkvp.tile([128, S], F32R, name=f"qT_{b}")
                kTb = qkvp.tile([128, S], F32R, name=f"kT_{b}")
                vnatb = [qkvp.tile([128, 128], F32R, name=f"v_{b}_{j}")
                         for j in range(N_JC)]
                # ---------- projections + rope for batch b ----------
                with (
                    tc.tile_pool(name=f"xt{b}", bufs=1) as xtp,
                ):
                    xt = xtp.tile([128, N_EC, S], F32R, name=f"xt_{b}")
                    xt_src = xT_d[b].ap().rearrange("(c p) s -> p c s", p=128)
                    for e in range(N_EC):
                        nc.sync.dma_start(xt[:, e, :], xt_src[:, e, :])

                    for w_sb, b_sb, dst in ((wq, bq, qTb), (wk, bk, kTb)):
                        for it in range(N_IT):
                            isl = slice(IT * it, IT * it + IT)
                            praw = psp.tile([128, IT], F32, tag="pqk")
                            for e in range(N_EC):
                                nc.tensor.matmul(
                                    praw[:], w_sb[:, e, :], xt[:, e, isl],
                                    start=(e == 0), stop=False)
                            nc.tensor.matmul(praw[:], b_sb[:], ones_row[:],
                                             start=False, stop=True)
                            raw = ptmp.tile([128, IT], F32R, tag="raw")
                            nc.vector.tensor_copy(raw[:], praw[:])
                            prot = psp.tile([128, IT], F32, tag="aux")
                            nc.tensor.matmul(prot[:], spt[:], raw[:],
                                             start=True, stop=True)
                            t1 = ptmp.tile([128, IT], F32, tag="t1")
                            nc.vector.tensor_tensor(t1[:], prot[:], sinM[:, isl], ALU.mult)
                            t2 = ptmp.tile([128, IT], F32, tag="t2")
                            nc.vector.tensor_tensor(
                                t2[:], raw[:].bitcast(F32), cosM[:, isl], ALU.mult)
                            nc.vector.tensor_tensor(dst[:, isl], t1[:], t2[:], ALU.add)

                    for it in range(N_IT):
                        isl = slice(IT * it, IT * it + IT)
                        pvt = psp.tile([128, IT], F32, tag="pqk")
                        for e in range(N_EC):
                            nc.tensor.matmul(pvt[:], wv[:, e, :], xt[:, e, isl],
                                             start=(e == 0), stop=False)
                        nc.tensor.matmul(pvt[:], bv[:], ones_row[:],
                                         start=False, stop=True)
                        vt_sb = ptmp.tile([128, IT], F32, tag="vt")
                        nc.vector.tensor_copy(vt_sb[:], pvt[:])
                        for jj in range(IT // 128):
                            jcc = it * (IT // 128) + jj
                            ptr_t = psp.tile([128, IT], F32, tag="aux", name="ptr_t")
                            ptr = ptr_t[:, 0:128]
                            nc.tensor.transpose(
                                ptr[:], vt_sb[:, 128 * jj : 128 * jj + 128], ident[:])
                            nc.vector.tensor_copy(vnatb[jcc][:], ptr[:])

                # ---------- attention for batch b ----------
                if True:
                    for it in range(N_IT):
                        isl = slice(IT * it, IT * it + IT)
                        pxv = [psa.tile([128, IT], F32, tag="xv", bufs=2, name=f"pxv{_h}") for _h in range(2)]
                        pD = [psa.tile([128, IT], F32, tag="D", bufs=2, name=f"pD{_h}") for _h in range(2)]
                        for jc in range(N_JC):
                            jsl = slice(JC * jc, JC * jc + JC)
                            psc = [psa.tile([128, IT], F32, tag="sc", bufs=2, name=f"psc{_h}") for _h in range(2)]
                            pt = [ptp.tile([128, IT], F32R, tag="pt", name=f"pt{_h}") for _h in range(2)]
                            for hh in range(2):
                                hsl = slice(64 * hh, 64 * hh + 64)
                                nc.tensor.matmul(psc[hh][:], kTb[hsl, jsl],
                                                 qTb[hsl, isl], start=True, stop=True)
                                nc.scalar.activation(pt[hh][:], psc[hh][:], AF.Exp,
                                                     scale=SCALE)
                            first, last = jc == 0, jc == N_JC - 1
                            for hh in range(2):
                                nc.tensor.matmul(
                                    pxv[hh][0:64, :],
                                    vnatb[jc][:, 64 * hh : 64 * hh + 64],
                                    pt[hh][:], start=first, stop=last)
                                nc.tensor.matmul(
                                    pD[hh][:], ones128[:], pt[hh][:],
                                    start=first, stop=last)
                        for hh in range(2):
                            rD = rp.tile([128, IT], F32, tag="rD")
                            nc.vector.reciprocal_approx_fast(rD[:], pD[hh][:])
                            nc.vector.tensor_tensor(
                                xvT[b][64 * hh : 64 * hh + 64, isl],
                                pxv[hh][0:64, :], rD[0:64, :], ALU.mult)

            # ---------- A2A ----------
            a2a_in = dram.tile([NCORES * 128, 512], F32)
            a2a_out = dram.tile([NCORES * 128, 512], F32)
            for j in range(NCORES):
                bj, blkj = j // 4, j % 4
                nc.sync.dma_start(
                    a2a_in[128 * j : 128 * j + 128, :],
                    xvT[bj][:, 512 * blkj : 512 * blkj + 512])
            if sim:
                # timing stand-in for TimelineSim (no collective support):
                # same-size DRAM->DRAM copy
                nc.sync.dma_start(a2a_out[:], a2a_in[:])
            else:
                nc.gpsimd.collective_compute(
                    "AllToAll", ALU.bypass,
                    replica_groups=[list(range(NCORES))],
                    ins=[a2a_in.opt()], outs=[a2a_out.opt()])

            # ---------- out-projection + layernorm ----------
            with (
                tc.tile_pool(name="wout_pool", bufs=1) as wp,
                tc.tile_pool(name="z_pool", bufs=1) as zp,
                tc.tile_pool(name="pso", bufs=2, space="PSUM") as pso,
                tc.tile_pool(name="ln_pool", bufs=2) as lnp,
            ):
                wout = wp.tile([128, N_EC, D], F32R, name="wout_sb")
                wout_src = wout_d.ap().rearrange("(c p) n -> p c n", p=128)
                for e in range(N_EC):
                    nc.sync.dma_start(wout[:, e, :], wout_src[:, e, :])
                bout = wp.tile([1, D], F32R, name="bout_sb")
                gamma = wp.tile([1, D], F32R, name="gamma_sb")
                beta = wp.tile([1, D], F32R, name="beta_sb")
                nc.sync.dma_start(bout[:], bout_d.ap())
                nc.sync.dma_start(gamma[:], gamma_d.ap())
                nc.sync.dma_start(beta[:], beta_d.ap())
                gbc = wp.tile([128, D], F32, name="gb_sb")
                bbc = wp.tile([128, D], F32, name="bb_sb")
                for half in range(2):
                    sl = slice(512 * half, 512 * half + 512)
                    pbc = pso.tile([128, 512], F32, tag="py", bufs=2)
                    nc.tensor.matmul(pbc[:], ones_col[:], gamma[:, sl],
                                     start=True, stop=True)
                    nc.scalar.copy(gbc[:, sl], pbc[:])
                    pbc2 = pso.tile([128, 512], F32, tag="py", bufs=2)
                    nc.tensor.matmul(pbc2[:], ones_col[:], beta[:, sl],
                                     start=True, stop=True)
                    nc.scalar.copy(bbc[:, sl], pbc2[:])

                eps_sb = zp.tile([128, 1], F32, name="eps_sb")
                nc.vector.memset(eps_sb[:], 1e-5)
                xres = [zp.tile([128, 512], F32, name=f"xres_{e}") for e in range(N_EC)]
                zT = [zp.tile([128, 512], F32R, name=f"zT_{e}") for e in range(N_EC)]
                for e in range(N_EC):
                    esl = slice(128 * e, 128 * e + 128)
                    nc.sync.dma_start(xres[e][:], xres_d.ap()[esl, :])
                    nc.sync.dma_start(zT[e][:], a2a_out[esl, :].bitcast(F32R))
                    nc.vector.tensor_tensor(
                        zT[e][:], zT[e][:].bitcast(F32), xres[e][:], ALU.add)

                for ic in range(4):
                    icl = slice(128 * ic, 128 * ic + 128)
                    py = [pso.tile([128, 512], F32, tag="py", bufs=2, name=f"py{_h}") for _h in range(2)]
                    for nh in range(2):
                        nsl = slice(512 * nh, 512 * nh + 512)
                        for e in range(N_EC):
                            nc.tensor.matmul(py[nh][:], zT[e][:, icl],
                                             wout[:, e, nsl],
                                             start=(e == 0), stop=False)
                        nc.tensor.matmul(py[nh][:], ones_col[:], bout[:, nsl],
                                         start=False, stop=True)
                    bn6 = lnp.tile([128, 2, 6], F32, tag="bn6")
                    nc.vector.bn_stats(bn6[:, 0, :], py[0][:])
                    nc.vector.bn_stats(bn6[:, 1, :], py[1][:])
                    bn2 = lnp.tile([128, 2], F32, tag="bn2")
                    nc.vector.bn_aggr(bn2[:], bn6[:])
                    lnv = lnp.tile([128, 1], F32, tag="lnv")
                    nc.scalar.activation(lnv[:], bn2[:, 1:2], AF.Ln, bias=eps_sb[:])
                    rstd = lnp.tile([128, 1], F32, tag="rstd")
                    nc.scalar.activation(rstd[:], lnv[:], AF.Exp, scale=-0.5)
                    yn = lnp.tile([128, D], F32, tag="yn")
                    for nh in range(2):
                        nsl = slice(512 * nh, 512 * nh + 512)
                        t = lnp.tile([128, 512], F32, tag="lt")
                        nc.vector.tensor_scalar(
                            t[:], py[nh][:], bn2[:, 0:1], rstd[:],
                            ALU.subtract, ALU.mult)
                        t2 = lnp.tile([128, 512], F32, tag="lt2")
                        nc.vector.tensor_tensor(t2[:], t[:], gbc[:, nsl], ALU.mult)
                        nc.vector.tensor_tensor(yn[:, nsl], t2[:], bbc[:, nsl], ALU.add)
                    nc.sync.dma_start(y_d.ap()[icl, :], yn[:])

    nc.compile()
    return nc


_NC_CACHE = None


def _get_nc():
    global _NC_CACHE
    if _NC_CACHE is None:
        _NC_CACHE = _build()
    return _NC_CACHE


def _prepare_in_maps(x, w_qkv, b_qkv, w_out, b_out, ln_gamma, ln_beta):
    x = np.asarray(x, dtype=np.float32)
    w_qkv = np.asarray(w_qkv, dtype=np.float32)
    b_qkv = np.asarray(b_qkv, dtype=np.float32)
    w_out = np.ascontiguousarray(np.asarray(w_out, dtype=np.float32))
    b_out = np.asarray(b_out, dtype=np.float32)
    ln_gamma = np.asarray(ln_gamma, dtype=np.float32)
    ln_beta = np.asarray(ln_beta, dtype=np.float32)

    cosM, sinM, SpermT = _rope_consts()
    ident = np.eye(128, dtype=np.float32)
    xT = [np.ascontiguousarray(x[b].T) for b in range(B)]

    in_maps = []
    for c in range(NCORES):
        h0 = 2 * c
        col = slice(HD * h0, HD * h0 + 128)
        myb, myblk = c // 4, c % 4
        m = {
            "xT0": xT[0], "xT1": xT[1],
            "wq": np.ascontiguousarray(w_qkv[:, col]),
            "wk": np.ascontiguousarray(w_qkv[:, D:][:, col]),
            "wv": np.ascontiguousarray(w_qkv[:, 2 * D:][:, col]),
            "bq": np.ascontiguousarray(b_qkv[col])[None, :],
            "bk": np.ascontiguousarray(b_qkv[D:][col])[None, :],
            "bv": np.ascontiguousarray(b_qkv[2 * D:][col])[None, :],
            "wout": w_out,
            "bout": b_out[None, :],
            "gamma": ln_gamma[None, :].astype(np.float32),
            "beta": ln_beta[None, :].astype(np.float32),
            "cosM": cosM, "sinM": sinM, "SpermT": SpermT, "ident": ident,
            "xres": np.ascontiguousarray(xT[myb][:, 512 * myblk : 512 * myblk + 512]),
        }
        in_maps.append(m)
    return in_maps


def _assemble(results):
    out = np.zeros((B, S, D), dtype=np.float32)
    for c in range(NCORES):
        myb, myblk = c // 4, c % 4
        out[myb, 512 * myblk : 512 * myblk + 512, :] = results[c]["y_out"]
    return out


def run(trace=False, **inputs):
    """Full run returning (output, BassKernelResults) — used by test.py for
    profiling; kernel() below is the graded entry point."""
    in_maps = _prepare_in_maps(**inputs)
    res = bass_utils.run_bass_kernel_spmd(
        _get_nc(), in_maps, core_ids=list(range(NCORES)), trace=trace)
    return _assemble(res.results), res


def kernel(**inputs):
    out, _ = run(trace=False, **inputs)
    return out


# revision 18
# speedup vs baseline: 1.1108x; 1.1108x over previous
"""Multi-head rotary attention block on 8 Trainium2 NeuronCores.

Sharding: tensor-parallel over heads (16 heads / 8 cores = 2 heads per core,
both batches on every core); one 8-way AllToAll redistributes the attention
output from head-sharded to sequence-sharded for the output projection, so
each core finishes layernorm on its own [512, 1024] output slice.

Per-core dataflow (feature-major "T" layouts are [channels, seq]):
  qT/kT = w_q^T x^T (+bias via K=1 matmul) with RoPE applied as
          raw*cosM + (SpermT^T raw)*sinM (rotation permutation as a matmul)
  vT    = w_v^T x^T, PE-transposed to natural v [seq, 128]
  per head: sT[j,i] = kT^T qT (transposed scores), pT = exp(sT/32) on ScalarE
  (no max subtraction: scores are O(0.5) under this operator's input law),
  softmax denominators D via ones-matmul over pT, 1/D = exp(-ln(D)) on ScalarE,
  xvT = (v^T pT) * (1/D); AllToAll; z = xv_gathered + x residual;
  y = z^T w_out + b_out; layernorm via bn_stats/bn_aggr + exp(-0.5 ln(var+eps)).

All matmuls run as float32r (full-rate fp32 PE mode; 4-byte data, producers
write into tiles declared float32r as the BIR verifier requires).
"""
import numpy as np

import concourse.bass as bass
import concourse.bacc as bacc
import concourse.tile as tile
import concourse.mybir as mybir
from concourse import bass_utils

F32 = mybir.dt.float32
F32R = mybir.dt.float32r
AF = mybir.ActivationFunctionType
ALU = mybir.AluOpType

NCORES = 8
B, S, D = 2, 2048, 1024
HEADS, HD = 16, 64
SCALE = 1.0 / float(np.sqrt(D))  # reference scales by full D, not head_dim
IT = 512          # i-tile width for attention
N_IT = S // IT    # 4
JC = 128          # j-chunk
N_JC = S // JC    # 16
N_EC = D // 128   # 8 e-chunks


def _rope_consts():
    rot = HD // 2
    inv_freq = 1.0 / (10000.0 ** (np.arange(0, rot, 2, dtype=np.float64) / rot))
    ang = np.arange(S, dtype=np.float64)[:, None] * inv_freq[None, :]
    ang = np.repeat(ang, 2, axis=-1)  # [S, 32]
    cos, sin = np.cos(ang), np.sin(ang)
    cosM = np.ones((128, S), dtype=np.float32)
    sinM = np.zeros((128, S), dtype=np.float32)
    for base in (0, 64):
        cosM[base : base + 32, :] = cos.T.astype(np.float32)
        sinM[base : base + 32, :] = sin.T.astype(np.float32)
    Sp = np.zeros((128, 128), dtype=np.float32)
    for base in (0, 64):
        for m in range(32):
            r0 = base + m
            if m % 2 == 0:
                Sp[r0, r0 + 1] = -1.0
            else:
                Sp[r0, r0 - 1] = 1.0
    SpermT = np.ascontiguousarray(Sp.T)
    return cosM, sinM, SpermT


def _build(sim=False):
    nc = bacc.Bacc("TRN2", target_bir_lowering=False, debug=False, num_devices=NCORES)

    xT_d = [nc.dram_tensor(f"xT{b}", [D, S], F32R, kind="ExternalInput") for b in range(B)]
    wq_d = nc.dram_tensor("wq", [D, 128], F32R, kind="ExternalInput")
    wk_d = nc.dram_tensor("wk", [D, 128], F32R, kind="ExternalInput")
    wv_d = nc.dram_tensor("wv", [D, 128], F32R, kind="ExternalInput")
    bq_d = nc.dram_tensor("bq", [1, 128], F32R, kind="ExternalInput")
    bk_d = nc.dram_tensor("bk", [1, 128], F32R, kind="ExternalInput")
    bv_d = nc.dram_tensor("bv", [1, 128], F32R, kind="ExternalInput")
    wout_d = nc.dram_tensor("wout", [D, D], F32R, kind="ExternalInput")
    bout_d = nc.dram_tensor("bout", [1, D], F32R, kind="ExternalInput")
    gamma_d = nc.dram_tensor("gamma", [1, D], F32R, kind="ExternalInput")
    beta_d = nc.dram_tensor("beta", [1, D], F32R, kind="ExternalInput")
    cosM_d = nc.dram_tensor("cosM", [128, S], F32, kind="ExternalInput")
    sinM_d = nc.dram_tensor("sinM", [128, S], F32, kind="ExternalInput")
    spt_d = nc.dram_tensor("SpermT", [128, 128], F32R, kind="ExternalInput")
    ident_d = nc.dram_tensor("ident", [128, 128], F32, kind="ExternalInput")
    xres_d = nc.dram_tensor("xres", [D, 512], F32, kind="ExternalInput")
    y_d = nc.dram_tensor("y_out", [512, D], F32, kind="ExternalOutput")

    with tile.TileContext(nc) as tc:
        with (
            tc.tile_pool(name="persist", bufs=1) as pp,
            tc.tile_pool(name="dram", bufs=1, space="DRAM") as dram,
        ):
            cosM = pp.tile([128, S], F32, name="cosM_sb")
            sinM = pp.tile([128, S], F32, name="sinM_sb")
            spt = pp.tile([128, 128], F32R, name="spt_sb")
            ident = pp.tile([128, 128], F32, name="ident_sb")
            nc.sync.dma_start(cosM[:], cosM_d.ap())
            nc.sync.dma_start(sinM[:], sinM_d.ap())
            nc.sync.dma_start(spt[:], spt_d.ap())
            nc.sync.dma_start(ident[:], ident_d.ap())

            wq = pp.tile([128, N_EC, 128], F32R, name="wq_sb")  # [p, ec, m]
            wk = pp.tile([128, N_EC, 128], F32R, name="wk_sb")
            wv = pp.tile([128, N_EC, 128], F32R, name="wv_sb")
            for w_sb, w_dd in ((wq, wq_d), (wk, wk_d), (wv, wv_d)):
                nc.sync.dma_start(
                    w_sb[:], w_dd.ap().rearrange("(c p) m -> p c m", p=128))
            bq = pp.tile([1, 128], F32R, name="bq_sb")
            bk = pp.tile([1, 128], F32R, name="bk_sb")
            bv = pp.tile([1, 128], F32R, name="bv_sb")
            nc.sync.dma_start(bq[:], bq_d.ap())
            nc.sync.dma_start(bk[:], bk_d.ap())
            nc.sync.dma_start(bv[:], bv_d.ap())
            ones_row = pp.tile([1, IT], F32R, name="ones_row")
            nc.vector.memset(ones_row[:].bitcast(F32), 1.0)
            ones128 = pp.tile([128, 128], F32R, name="ones128")
            nc.vector.memset(ones128[:].bitcast(F32), 1.0)
            ones_col = pp.tile([1, 128], F32R, name="ones_col")
            nc.vector.memset(ones_col[:].bitcast(F32), 1.0)

            xvT = [pp.tile([128, S], F32, name=f"xvT_{b}") for b in range(B)]

            with (
                tc.tile_pool(name="psp", bufs=1, space="PSUM") as psp,
                tc.tile_pool(name="psa", bufs=1, space="PSUM") as psa,
                tc.tile_pool(name="ptmp", bufs=3) as ptmp,
                tc.tile_pool(name="ptp", bufs=6) as ptp,
                tc.tile_pool(name="rp", bufs=3) as rp,
            ):
             for b in range(B):
              with tc.tile_pool(name=f"qkv{b}", bufs=1) as qkvp:
                qTb = qkvp.tile([128, S], F32R, name=f"qT_{b}")
                kTb = qkvp.tile([128, S], F32R, name=f"kT_{b}")
                vnatb = [qkvp.tile([128, 130], F32R, name=f"v_{b}_{j}")
                         for j in range(N_JC)]
                for j in range(N_JC):
                    nc.vector.memset(vnatb[j][:, 64:65].bitcast(F32), 1.0)
                    nc.vector.memset(vnatb[j][:, 129:130].bitcast(F32), 1.0)
                # ---------- projections + rope for batch b ----------
                with (
                    tc.tile_pool(name=f"xt{b}", bufs=1) as xtp,
                ):
                    xt = xtp.tile([128, N_EC, S], F32R, name=f"xt_{b}")
                    xt_src = xT_d[b].ap().rearrange("(c p) s -> p c s", p=128)
                    for e in range(N_EC):
                        nc.sync.dma_start(xt[:, e, :], xt_src[:, e, :])

                    for w_sb, b_sb, dst in ((wq, bq, qTb), (wk, bk, kTb)):
                        for it in range(N_IT):
                            isl = slice(IT * it, IT * it + IT)
                            praw = psp.tile([128, IT], F32, tag="pqk")
                            for e in range(N_EC):
                                nc.tensor.matmul(
                                    praw[:], w_sb[:, e, :], xt[:, e, isl],
                                    start=(e == 0), stop=False)
                            nc.tensor.matmul(praw[:], b_sb[:], ones_row[:],
                                             start=False, stop=True)
                            raw = ptmp.tile([128, IT], F32R, tag="raw")
                            nc.vector.tensor_copy(raw[:], praw[:])
                            prot = psp.tile([128, IT], F32, tag="aux")
                            nc.tensor.matmul(prot[:], spt[:], raw[:],
                                             start=True, stop=True)
                            t1 = ptmp.tile([128, IT], F32, tag="t1")
                            nc.vector.tensor_tensor(t1[:], prot[:], sinM[:, isl], ALU.mult)
                            t2 = ptmp.tile([128, IT], F32, tag="t2")
                            nc.vector.tensor_tensor(
                                t2[:], raw[:].bitcast(F32), cosM[:, isl], ALU.mult)
                            nc.vector.tensor_tensor(dst[:, isl], t1[:], t2[:], ALU.add)

                    for it in range(N_IT):
                        isl = slice(IT * it, IT * it + IT)
                        pvt = psp.tile([128, IT], F32, tag="pqk")
                        for e in range(N_EC):
                            nc.tensor.matmul(pvt[:], wv[:, e, :], xt[:, e, isl],
                                             start=(e == 0), stop=False)
                        nc.tensor.matmul(pvt[:], bv[:], ones_row[:],
                                         start=False, stop=True)
                        vt_sb = ptmp.tile([128, IT], F32, tag="vt")
                        nc.vector.tensor_copy(vt_sb[:], pvt[:])
                        for jj in range(IT // 128):
                            jcc = it * (IT // 128) + jj
                            ptr_t = psp.tile([128, IT], F32, tag="aux", name="ptr_t")
                            ptr = ptr_t[:, 0:128]
                            nc.tensor.transpose(
                                ptr[:], vt_sb[:, 128 * jj : 128 * jj + 128], ident[:])
                            nc.vector.tensor_copy(vnatb[jcc][:, 0:64], ptr[:, 0:64])
                            nc.vector.tensor_copy(vnatb[jcc][:, 65:129], ptr[:, 64:128])

                # ---------- attention for batch b ----------
                if True:
                    for it in range(N_IT):
                        isl = slice(IT * it, IT * it + IT)
                        pxv = [psa.tile([128, IT], F32, tag="xv", bufs=2, name=f"pxv{_h}") for _h in range(2)]
                        for jc in range(N_JC):
                            jsl = slice(JC * jc, JC * jc + JC)
                            psc = [psa.tile([128, IT], F32, tag="sc", bufs=3, name=f"psc{_h}") for _h in range(2)]
                            pt = [ptp.tile([128, IT], F32R, tag="pt", name=f"pt{_h}") for _h in range(2)]
                            for hh in range(2):
                                hsl = slice(64 * hh, 64 * hh + 64)
                                nc.tensor.matmul(psc[hh][:], kTb[hsl, jsl],
                                                 qTb[hsl, isl], start=True, stop=True)
                                nc.scalar.activation(pt[hh][:], psc[hh][:], AF.Exp,
                                                     scale=SCALE)
                            first, last = jc == 0, jc == N_JC - 1
                            for hh in range(2):
                                nc.tensor.matmul(
                                    pxv[hh][0:65, :],
                                    vnatb[jc][:, 65 * hh : 65 * hh + 65],
                                    pt[hh][:], start=first, stop=last)
                        for hh in range(2):
                            rDf = rp.tile([128, IT], F32, tag="rDf")
                            nc.vector.reciprocal_approx_fast(
                                rDf[64:65, :], pxv[hh][64:65, :])
                            rD = rp.tile([128, IT], F32R, tag="rD")
                            nc.vector.tensor_copy(rD[64:65, :], rDf[64:65, :])
                            rDb = psa.tile([128, IT], F32, tag="rdb", bufs=1, name="rDb")
                            nc.tensor.matmul(rDb[0:64, :], ones128[64:65, 0:64],
                                             rD[64:65, :], start=True, stop=True)
                            rDs = rp.tile([128, IT], F32, tag="rDs")
                            nc.vector.tensor_copy(rDs[0:64, :], rDb[0:64, :])
                            nc.vector.tensor_tensor(
                                xvT[b][64 * hh : 64 * hh + 64, isl],
                                pxv[hh][0:64, :], rDs[0:64, :], ALU.mult)

            # ---------- A2A ----------
            a2a_in = dram.tile([NCORES * 128, 512], F32)
            a2a_out = dram.tile([NCORES * 128, 512], F32)
            for j in range(NCORES):
                bj, blkj = j // 4, j % 4
                nc.sync.dma_start(
                    a2a_in[128 * j : 128 * j + 128, :],
                    xvT[bj][:, 512 * blkj : 512 * blkj + 512])
            if sim:
                # timing stand-in for TimelineSim (no collective support):
                # same-size DRAM->DRAM copy
                nc.sync.dma_start(a2a_out[:], a2a_in[:])
            else:
                nc.gpsimd.collective_compute(
                    "AllToAll", ALU.bypass,
                    replica_groups=[list(range(NCORES))],
                    ins=[a2a_in.opt()], outs=[a2a_out.opt()])

            # ---------- out-projection + layernorm ----------
            with (
                tc.tile_pool(name="wout_pool", bufs=1) as wp,
                tc.tile_pool(name="z_pool", bufs=1) as zp,
                tc.tile_pool(name="pso", bufs=2, space="PSUM") as pso,
                tc.tile_pool(name="ln_pool", bufs=2) as lnp,
            ):
                wout = wp.tile([128, N_EC, D], F32R, name="wout_sb")
                wout_src = wout_d.ap().rearrange("(c p) n -> p c n", p=128)
                for e in range(N_EC):
                    nc.sync.dma_start(wout[:, e, :], wout_src[:, e, :])
                bout = wp.tile([1, D], F32R, name="bout_sb")
                gamma = wp.tile([1, D], F32R, name="gamma_sb")
                beta = wp.tile([1, D], F32R, name="beta_sb")
                nc.sync.dma_start(bout[:], bout_d.ap())
                nc.sync.dma_start(gamma[:], gamma_d.ap())
                nc.sync.dma_start(beta[:], beta_d.ap())
                gbc = wp.tile([128, D], F32, name="gb_sb")
                bbc = wp.tile([128, D], F32, name="bb_sb")
                for half in range(2):
                    sl = slice(512 * half, 512 * half + 512)
                    pbc = pso.tile([128, 512], F32, tag="py", bufs=2)
                    nc.tensor.matmul(pbc[:], ones_col[:], gamma[:, sl],
                                     start=True, stop=True)
                    nc.scalar.copy(gbc[:, sl], pbc[:])
                    pbc2 = pso.tile([128, 512], F32, tag="py", bufs=2)
                    nc.tensor.matmul(pbc2[:], ones_col[:], beta[:, sl],
                                     start=True, stop=True)
                    nc.scalar.copy(bbc[:, sl], pbc2[:])

                eps_sb = zp.tile([128, 1], F32, name="eps_sb")
                nc.vector.memset(eps_sb[:], 1e-5)
                xres = [zp.tile([128, 512], F32, name=f"xres_{e}") for e in range(N_EC)]
                zT = [zp.tile([128, 512], F32R, name=f"zT_{e}") for e in range(N_EC)]
                for e in range(N_EC):
                    esl = slice(128 * e, 128 * e + 128)
                    nc.sync.dma_start(xres[e][:], xres_d.ap()[esl, :])
                    nc.sync.dma_start(zT[e][:], a2a_out[esl, :].bitcast(F32R))
                    nc.vector.tensor_tensor(
                        zT[e][:], zT[e][:].bitcast(F32), xres[e][:], ALU.add)

                for ic in range(4):
                    icl = slice(128 * ic, 128 * ic + 128)
                    py = [pso.tile([128, 512], F32, tag="py", bufs=2, name=f"py{_h}") for _h in range(2)]
                    for nh in range(2):
                        nsl = slice(512 * nh, 512 * nh + 512)
                        for e in range(N_EC):
                            nc.tensor.matmul(py[nh][:], zT[e][:, icl],
                                             wout[:, e, nsl],
                                             start=(e == 0), stop=False)
                        nc.tensor.matmul(py[nh][:], ones_col[:], bout[:, nsl],
                                         start=False, stop=True)
                    bn6 = lnp.tile([128, 2, 6], F32, tag="bn6")
                    nc.vector.bn_stats(bn6[:, 0, :], py[0][:])
                    nc.vector.bn_stats(bn6[:, 1, :], py[1][:])
                    bn2 = lnp.tile([128, 2], F32, tag="bn2")
                    nc.vector.bn_aggr(bn2[:], bn6[:])
                    lnv = lnp.tile([128, 1], F32, tag="lnv")
                    nc.scalar.activation(lnv[:], bn2[:, 1:2], AF.Ln, bias=eps_sb[:])
                    rstd = lnp.tile([128, 1], F32, tag="rstd")
                    nc.scalar.activation(rstd[:], lnv[:], AF.Exp, scale=-0.5)
                    yn = lnp.tile([128, D], F32, tag="yn")
                    for nh in range(2):
                        nsl = slice(512 * nh, 512 * nh + 512)
                        t = lnp.tile([128, 512], F32, tag="lt")
                        nc.vector.tensor_scalar(
                            t[:], py[nh][:], bn2[:, 0:1], rstd[:],
                            ALU.subtract, ALU.mult)
                        t2 = lnp.tile([128, 512], F32, tag="lt2")
                        nc.vector.tensor_tensor(t2[:], t[:], gbc[:, nsl], ALU.mult)
                        nc.vector.tensor_tensor(yn[:, nsl], t2[:], bbc[:, nsl], ALU.add)
                    nc.sync.dma_start(y_d.ap()[icl, :], yn[:])

    nc.compile()
    return nc


_NC_CACHE = None


def _get_nc():
    global _NC_CACHE
    if _NC_CACHE is None:
        _NC_CACHE = _build()
    return _NC_CACHE


def _prepare_in_maps(x, w_qkv, b_qkv, w_out, b_out, ln_gamma, ln_beta):
    x = np.asarray(x, dtype=np.float32)
    w_qkv = np.asarray(w_qkv, dtype=np.float32)
    b_qkv = np.asarray(b_qkv, dtype=np.float32)
    w_out = np.ascontiguousarray(np.asarray(w_out, dtype=np.float32))
    b_out = np.asarray(b_out, dtype=np.float32)
    ln_gamma = np.asarray(ln_gamma, dtype=np.float32)
    ln_beta = np.asarray(ln_beta, dtype=np.float32)

    cosM, sinM, SpermT = _rope_consts()
    ident = np.eye(128, dtype=np.float32)
    xT = [np.ascontiguousarray(x[b].T) for b in range(B)]

    in_maps = []
    for c in range(NCORES):
        h0 = 2 * c
        col = slice(HD * h0, HD * h0 + 128)
        myb, myblk = c // 4, c % 4
        m = {
            "xT0": xT[0], "xT1": xT[1],
            "wq": np.ascontiguousarray(w_qkv[:, col]),
            "wk": np.ascontiguousarray(w_qkv[:, D:][:, col]),
            "wv": np.ascontiguousarray(w_qkv[:, 2 * D:][:, col]),
            "bq": np.ascontiguousarray(b_qkv[col])[None, :],
            "bk": np.ascontiguousarray(b_qkv[D:][col])[None, :],
            "bv": np.ascontiguousarray(b_qkv[2 * D:][col])[None, :],
            "wout": w_out,
            "bout": b_out[None, :],
            "gamma": ln_gamma[None, :].astype(np.float32),
            "beta": ln_beta[None, :].astype(np.float32),
            "cosM": cosM, "sinM": sinM, "SpermT": SpermT, "ident": ident,
            "xres": np.ascontiguousarray(xT[myb][:, 512 * myblk : 512 * myblk + 512]),
        }
        in_maps.append(m)
    return in_maps


def _assemble(results):
    out = np.zeros((B, S, D), dtype=np.float32)
    for c in range(NCORES):
        myb, myblk = c // 4, c % 4
        out[myb, 512 * myblk : 512 * myblk + 512, :] = results[c]["y_out"]
    return out


def run(trace=False, **inputs):
    """Full run returning (output, BassKernelResults) — used by test.py for
    profiling; kernel() below is the graded entry point."""
    in_maps = _prepare_in_maps(**inputs)
    res = bass_utils.run_bass_kernel_spmd(
        _get_nc(), in_maps, core_ids=list(range(NCORES)), trace=trace)
    return _assemble(res.results), res


def kernel(**inputs):
    out, _ = run(trace=False, **inputs)
    return out


# revision 22
# speedup vs baseline: 1.1110x; 1.0001x over previous
"""Multi-head rotary attention block on 8 Trainium2 NeuronCores.

Sharding: tensor-parallel over heads (16 heads / 8 cores = 2 heads per core,
both batches on every core); one 8-way AllToAll redistributes the attention
output from head-sharded to sequence-sharded for the output projection, so
each core finishes layernorm on its own [512, 1024] output slice.

Per-core dataflow (feature-major "T" layouts are [channels, seq]):
  qT/kT = w_q^T x^T (+bias via K=1 matmul) with RoPE applied as
          raw*cosM + (SpermT^T raw)*sinM (rotation permutation as a matmul)
  vT    = w_v^T x^T, PE-transposed to natural v [seq, 128]
  per head: sT[j,i] = kT^T qT (transposed scores), pT = exp(sT/32) on ScalarE
  (no max subtraction: scores are O(0.5) under this operator's input law),
  softmax denominators D via ones-matmul over pT, 1/D = exp(-ln(D)) on ScalarE,
  xvT = (v^T pT) * (1/D); AllToAll; z = xv_gathered + x residual;
  y = z^T w_out + b_out; layernorm via bn_stats/bn_aggr + exp(-0.5 ln(var+eps)).

All matmuls run as float32r (full-rate fp32 PE mode; 4-byte data, producers
write into tiles declared float32r as the BIR verifier requires).
"""
import numpy as np

import concourse.bass as bass
import concourse.bacc as bacc
import concourse.tile as tile
import concourse.mybir as mybir
from concourse import bass_utils

F32 = mybir.dt.float32
F32R = mybir.dt.float32r
AF = mybir.ActivationFunctionType
ALU = mybir.AluOpType

NCORES = 8
B, S, D = 2, 2048, 1024
HEADS, HD = 16, 64
SCALE = 1.0 / float(np.sqrt(D))  # reference scales by full D, not head_dim
IT = 512          # i-tile width for attention
N_IT = S // IT    # 4
JC = 128          # j-chunk
N_JC = S // JC    # 16
N_EC = D // 128   # 8 e-chunks


def _rope_consts():
    rot = HD // 2
    inv_freq = 1.0 / (10000.0 ** (np.arange(0, rot, 2, dtype=np.float64) / rot))
    ang = np.arange(S, dtype=np.float64)[:, None] * inv_freq[None, :]
    ang = np.repeat(ang, 2, axis=-1)  # [S, 32]
    cos, sin = np.cos(ang), np.sin(ang)
    cosM = np.ones((128, S), dtype=np.float32)
    sinM = np.zeros((128, S), dtype=np.float32)
    for base in (0, 64):
        cosM[base : base + 32, :] = cos.T.astype(np.float32)
        sinM[base : base + 32, :] = sin.T.astype(np.float32)
    Sp = np.zeros((128, 128), dtype=np.float32)
    for base in (0, 64):
        for m in range(32):
            r0 = base + m
            if m % 2 == 0:
                Sp[r0, r0 + 1] = -1.0
            else:
                Sp[r0, r0 - 1] = 1.0
    SpermT = np.ascontiguousarray(Sp.T)
    return cosM, sinM, SpermT


def _build(sim=False):
    nc = bacc.Bacc("TRN2", target_bir_lowering=False, debug=False, num_devices=NCORES)

    xT_d = [nc.dram_tensor(f"xT{b}", [D, S], F32R, kind="ExternalInput") for b in range(B)]
    wq_d = nc.dram_tensor("wq", [D, 128], F32R, kind="ExternalInput")
    wk_d = nc.dram_tensor("wk", [D, 128], F32R, kind="ExternalInput")
    wv_d = nc.dram_tensor("wv", [D, 128], F32R, kind="ExternalInput")
    bq_d = nc.dram_tensor("bq", [1, 128], F32R, kind="ExternalInput")
    bk_d = nc.dram_tensor("bk", [1, 128], F32R, kind="ExternalInput")
    bv_d = nc.dram_tensor("bv", [1, 128], F32R, kind="ExternalInput")
    wout_d = nc.dram_tensor("wout", [D, D], F32R, kind="ExternalInput")
    bout_d = nc.dram_tensor("bout", [1, D], F32R, kind="ExternalInput")
    gamma_d = nc.dram_tensor("gamma", [1, D], F32R, kind="ExternalInput")
    beta_d = nc.dram_tensor("beta", [1, D], F32R, kind="ExternalInput")
    cosM_d = nc.dram_tensor("cosM", [128, S], F32, kind="ExternalInput")
    sinM_d = nc.dram_tensor("sinM", [128, S], F32, kind="ExternalInput")
    spt_d = nc.dram_tensor("SpermT", [128, 128], F32R, kind="ExternalInput")
    ident_d = nc.dram_tensor("ident", [128, 128], F32, kind="ExternalInput")
    xres_d = nc.dram_tensor("xres", [D, 512], F32, kind="ExternalInput")
    y_d = nc.dram_tensor("y_out", [512, D], F32, kind="ExternalOutput")

    with tile.TileContext(nc) as tc:
        with (
            tc.tile_pool(name="persist", bufs=1) as pp,
            tc.tile_pool(name="dram", bufs=1, space="DRAM") as dram,
        ):
            cosM = pp.tile([128, S], F32, name="cosM_sb")
            sinM = pp.tile([128, S], F32, name="sinM_sb")
            spt = pp.tile([128, 128], F32R, name="spt_sb")
            ident = pp.tile([128, 128], F32, name="ident_sb")
            nc.sync.dma_start(cosM[:], cosM_d.ap())
            nc.sync.dma_start(sinM[:], sinM_d.ap())
            nc.sync.dma_start(spt[:], spt_d.ap())
            nc.sync.dma_start(ident[:], ident_d.ap())

            wq = pp.tile([128, N_EC, 128], F32R, name="wq_sb")  # [p, ec, m]
            wk = pp.tile([128, N_EC, 128], F32R, name="wk_sb")
            wv = pp.tile([128, N_EC, 128], F32R, name="wv_sb")
            for w_sb, w_dd in ((wq, wq_d), (wk, wk_d), (wv, wv_d)):
                nc.sync.dma_start(
                    w_sb[:], w_dd.ap().rearrange("(c p) m -> p c m", p=128))
            bq = pp.tile([1, 128], F32R, name="bq_sb")
            bk = pp.tile([1, 128], F32R, name="bk_sb")
            bv = pp.tile([1, 128], F32R, name="bv_sb")
            nc.sync.dma_start(bq[:], bq_d.ap())
            nc.sync.dma_start(bk[:], bk_d.ap())
            nc.sync.dma_start(bv[:], bv_d.ap())
            ones_row = pp.tile([1, IT], F32R, name="ones_row")
            nc.vector.memset(ones_row[:].bitcast(F32), 1.0)
            ones128 = pp.tile([128, 128], F32R, name="ones128")
            nc.vector.memset(ones128[:].bitcast(F32), 1.0)
            ones_col = pp.tile([1, 128], F32R, name="ones_col")
            nc.vector.memset(ones_col[:].bitcast(F32), 1.0)

            xvT = [pp.tile([128, S], F32, name=f"xvT_{b}") for b in range(B)]

            with (
                tc.tile_pool(name="psp", bufs=1, space="PSUM") as psp,
                tc.tile_pool(name="psa", bufs=1, space="PSUM") as psa,
                tc.tile_pool(name="ptmp", bufs=3) as ptmp,
                tc.tile_pool(name="ptp", bufs=6) as ptp,
                tc.tile_pool(name="rp", bufs=3) as rp,
            ):
             for b in range(B):
              with tc.tile_pool(name=f"qkv{b}", bufs=1) as qkvp:
                qTb = qkvp.tile([128, S], F32R, name=f"qT_{b}")
                kTb = qkvp.tile([128, S], F32R, name=f"kT_{b}")
                vnatb = [qkvp.tile([128, 130], F32R, name=f"v_{b}_{j}")
                         for j in range(N_JC)]
                for j in range(N_JC):
                    nc.vector.memset(vnatb[j][:, 64:65].bitcast(F32), 1.0)
                    nc.vector.memset(vnatb[j][:, 129:130].bitcast(F32), 1.0)
                # ---------- projections + rope for batch b ----------
                with (
                    tc.tile_pool(name=f"xt{b}", bufs=1) as xtp,
                ):
                    xt = xtp.tile([128, N_EC, S], F32R, name=f"xt_{b}")
                    xt_src = xT_d[b].ap().rearrange("(c p) s -> p c s", p=128)
                    for e in range(N_EC):
                        nc.sync.dma_start(xt[:, e, :], xt_src[:, e, :])

                    for w_sb, b_sb, dst in ((wq, bq, qTb), (wk, bk, kTb)):
                        for it in range(N_IT):
                            isl = slice(IT * it, IT * it + IT)
                            praw = psp.tile([128, IT], F32, tag="pqk")
                            for e in range(N_EC):
                                nc.tensor.matmul(
                                    praw[:], w_sb[:, e, :], xt[:, e, isl],
                                    start=(e == 0), stop=False)
                            nc.tensor.matmul(praw[:], b_sb[:], ones_row[:],
                                             start=False, stop=True)
                            raw = ptmp.tile([128, IT], F32R, tag="raw")
                            nc.vector.tensor_copy(raw[:], praw[:])
                            prot = psp.tile([128, IT], F32, tag="aux")
                            nc.tensor.matmul(prot[:], spt[:], raw[:],
                                             start=True, stop=True)
                            t1 = ptmp.tile([128, IT], F32, tag="t1")
                            nc.vector.tensor_tensor(t1[:], prot[:], sinM[:, isl], ALU.mult)
                            t2 = ptmp.tile([128, IT], F32, tag="t2")
                            nc.vector.tensor_tensor(
                                t2[:], raw[:].bitcast(F32), cosM[:, isl], ALU.mult)
                            nc.vector.tensor_tensor(dst[:, isl], t1[:], t2[:], ALU.add)

                    for it in range(N_IT):
                        isl = slice(IT * it, IT * it + IT)
                        pvt = psp.tile([128, IT], F32, tag="pqk")
                        for e in range(N_EC):
                            nc.tensor.matmul(pvt[:], wv[:, e, :], xt[:, e, isl],
                                             start=(e == 0), stop=False)
                        nc.tensor.matmul(pvt[:], bv[:], ones_row[:],
                                         start=False, stop=True)
                        vt_sb = ptmp.tile([128, IT], F32, tag="vt")
                        nc.vector.tensor_copy(vt_sb[:], pvt[:])
                        for jj in range(IT // 128):
                            jcc = it * (IT // 128) + jj
                            ptr_t = psp.tile([128, IT], F32, tag="aux", name="ptr_t")
                            ptr = ptr_t[:, 0:128]
                            nc.tensor.transpose(
                                ptr[:], vt_sb[:, 128 * jj : 128 * jj + 128], ident[:])
                            nc.vector.tensor_copy(vnatb[jcc][:, 0:64], ptr[:, 0:64])
                            nc.vector.tensor_copy(vnatb[jcc][:, 65:129], ptr[:, 64:128])

                # ---------- attention for batch b ----------
                if True:
                    for it in range(N_IT):
                        isl = slice(IT * it, IT * it + IT)
                        pxv = [psa.tile([128, IT], F32, tag="xv", bufs=2, name=f"pxv{_h}") for _h in range(2)]
                        for jc in range(N_JC):
                            jsl = slice(JC * jc, JC * jc + JC)
                            psc = [psa.tile([128, IT], F32, tag="sc", bufs=3, name=f"psc{_h}") for _h in range(2)]
                            pt = [ptp.tile([128, IT], F32R, tag="pt", name=f"pt{_h}") for _h in range(2)]
                            for hh in range(2):
                                hsl = slice(64 * hh, 64 * hh + 64)
                                nc.tensor.matmul(psc[hh][:], kTb[hsl, jsl],
                                                 qTb[hsl, isl], start=True, stop=True)
                                nc.scalar.activation(pt[hh][:], psc[hh][:], AF.Exp,
                                                     scale=SCALE)
                            first, last = jc == 0, jc == N_JC - 1
                            for hh in range(2):
                                nc.tensor.matmul(
                                    pxv[hh][0:65, :],
                                    vnatb[jc][:, 65 * hh : 65 * hh + 65],
                                    pt[hh][:], start=first, stop=last)
                        for hh in range(2):
                            rDf = rp.tile([128, IT], F32, tag="rDf")
                            nc.vector.reciprocal_approx_fast(
                                rDf[0:65, :], pxv[hh][0:65, :])
                            rD = rp.tile([128, IT], F32R, tag="rD")
                            nc.vector.tensor_copy(rD[0:1, :], rDf[64:65, :])
                            rDb = psa.tile([128, IT], F32, tag="rdb", bufs=1, name="rDb")
                            nc.tensor.matmul(rDb[0:64, :], ones_col[:, 0:64],
                                             rD[0:1, :], start=True, stop=True)
                            rDs = rp.tile([128, IT], F32, tag="rDs")
                            nc.vector.tensor_copy(rDs[0:64, :], rDb[0:64, :])
                            nc.vector.tensor_tensor(
                                xvT[b][64 * hh : 64 * hh + 64, isl],
                                pxv[hh][0:64, :], rDs[0:64, :], ALU.mult)

            # ---------- A2A ----------
            a2a_in = dram.tile([NCORES * 128, 512], F32)
            a2a_out = dram.tile([NCORES * 128, 512], F32)
            for j in range(NCORES):
                bj, blkj = j // 4, j % 4
                nc.sync.dma_start(
                    a2a_in[128 * j : 128 * j + 128, :],
                    xvT[bj][:, 512 * blkj : 512 * blkj + 512])
            if sim:
                # timing stand-in for TimelineSim (no collective support):
                # same-size DRAM->DRAM copy
                nc.sync.dma_start(a2a_out[:], a2a_in[:])
            else:
                nc.gpsimd.collective_compute(
                    "AllToAll", ALU.bypass,
                    replica_groups=[list(range(NCORES))],
                    ins=[a2a_in.opt()], outs=[a2a_out.opt()])

            # ---------- out-projection + layernorm ----------
            with (
                tc.tile_pool(name="wout_pool", bufs=1) as wp,
                tc.tile_pool(name="z_pool", bufs=1) as zp,
                tc.tile_pool(name="pso", bufs=2, space="PSUM") as pso,
                tc.tile_pool(name="ln_pool", bufs=2) as lnp,
            ):
                wout = wp.tile([128, N_EC, D], F32R, name="wout_sb")
                wout_src = wout_d.ap().rearrange("(c p) n -> p c n", p=128)
                for e in range(N_EC):
                    nc.sync.dma_start(wout[:, e, :], wout_src[:, e, :])
                bout = wp.tile([1, D], F32R, name="bout_sb")
                gamma = wp.tile([1, D], F32R, name="gamma_sb")
                beta = wp.tile([1, D], F32R, name="beta_sb")
                nc.sync.dma_start(bout[:], bout_d.ap())
                nc.sync.dma_start(gamma[:], gamma_d.ap())
                nc.sync.dma_start(beta[:], beta_d.ap())
                gbc = wp.tile([128, D], F32, name="gb_sb")
                bbc = wp.tile([128, D], F32, name="bb_sb")
                for half in range(2):
                    sl = slice(512 * half, 512 * half + 512)
                    pbc = pso.tile([128, 512], F32, tag="py", bufs=2)
                    nc.tensor.matmul(pbc[:], ones_col[:], gamma[:, sl],
                                     start=True, stop=True)
                    nc.scalar.copy(gbc[:, sl], pbc[:])
                    pbc2 = pso.tile([128, 512], F32, tag="py", bufs=2)
                    nc.tensor.matmul(pbc2[:], ones_col[:], beta[:, sl],
                                     start=True, stop=True)
                    nc.scalar.copy(bbc[:, sl], pbc2[:])

                eps_sb = zp.tile([128, 1], F32, name="eps_sb")
                nc.vector.memset(eps_sb[:], 1e-5)
                xres = [zp.tile([128, 512], F32, name=f"xres_{e}") for e in range(N_EC)]
                zT = [zp.tile([128, 512], F32R, name=f"zT_{e}") for e in range(N_EC)]
                for e in range(N_EC):
                    esl = slice(128 * e, 128 * e + 128)
                    nc.sync.dma_start(xres[e][:], xres_d.ap()[esl, :])
                    nc.sync.dma_start(zT[e][:], a2a_out[esl, :].bitcast(F32R))
                    nc.vector.tensor_tensor(
                        zT[e][:], zT[e][:].bitcast(F32), xres[e][:], ALU.add)

                for ic in range(4):
                    icl = slice(128 * ic, 128 * ic + 128)
                    py = [pso.tile([128, 512], F32, tag="py", bufs=2, name=f"py{_h}") for _h in range(2)]
                    for nh in range(2):
                        nsl = slice(512 * nh, 512 * nh + 512)
                        for e in range(N_EC):
                            nc.tensor.matmul(py[nh][:], zT[e][:, icl],
                                             wout[:, e, nsl],
                                             start=(e == 0), stop=False)
                        nc.tensor.matmul(py[nh][:], ones_col[:], bout[:, nsl],
                                         start=False, stop=True)
                    bn6 = lnp.tile([128, 2, 6], F32, tag="bn6")
                    nc.vector.bn_stats(bn6[:, 0, :], py[0][:])
                    nc.vector.bn_stats(bn6[:, 1, :], py[1][:])
                    bn2 = lnp.tile([128, 2], F32, tag="bn2")
                    nc.vector.bn_aggr(bn2[:], bn6[:])
                    lnv = lnp.tile([128, 1], F32, tag="lnv")
                    nc.scalar.activation(lnv[:], bn2[:, 1:2], AF.Ln, bias=eps_sb[:])
                    rstd = lnp.tile([128, 1], F32, tag="rstd")
                    nc.scalar.activation(rstd[:], lnv[:], AF.Exp, scale=-0.5)
                    yn = lnp.tile([128, D], F32, tag="yn")
                    for nh in range(2):
                        nsl = slice(512 * nh, 512 * nh + 512)
                        t = lnp.tile([128, 512], F32, tag="lt")
                        nc.vector.tensor_scalar(
                            t[:], py[nh][:], bn2[:, 0:1], rstd[:],
                            ALU.subtract, ALU.mult)
                        t2 = lnp.tile([128, 512], F32, tag="lt2")
                        nc.vector.tensor_tensor(t2[:], t[:], gbc[:, nsl], ALU.mult)
                        nc.vector.tensor_tensor(yn[:, nsl], t2[:], bbc[:, nsl], ALU.add)
                    nc.sync.dma_start(y_d.ap()[icl, :], yn[:])

    nc.compile()
    return nc


_NC_CACHE = None


def _get_nc():
    global _NC_CACHE
    if _NC_CACHE is None:
        _NC_CACHE = _build()
    return _NC_CACHE


def _prepare_in_maps(x, w_qkv, b_qkv, w_out, b_out, ln_gamma, ln_beta):
    x = np.asarray(x, dtype=np.float32)
    w_qkv = np.asarray(w_qkv, dtype=np.float32)
    b_qkv = np.asarray(b_qkv, dtype=np.float32)
    w_out = np.ascontiguousarray(np.asarray(w_out, dtype=np.float32))
    b_out = np.asarray(b_out, dtype=np.float32)
    ln_gamma = np.asarray(ln_gamma, dtype=np.float32)
    ln_beta = np.asarray(ln_beta, dtype=np.float32)

    cosM, sinM, SpermT = _rope_consts()
    ident = np.eye(128, dtype=np.float32)
    xT = [np.ascontiguousarray(x[b].T) for b in range(B)]

    in_maps = []
    for c in range(NCORES):
        h0 = 2 * c
        col = slice(HD * h0, HD * h0 + 128)
        myb, myblk = c // 4, c % 4
        m = {
            "xT0": xT[0], "xT1": xT[1],
            "wq": np.ascontiguousarray(w_qkv[:, col]),
            "wk": np.ascontiguousarray(w_qkv[:, D:][:, col]),
            "wv": np.ascontiguousarray(w_qkv[:, 2 * D:][:, col]),
            "bq": np.ascontiguousarray(b_qkv[col])[None, :],
            "bk": np.ascontiguousarray(b_qkv[D:][col])[None, :],
            "bv": np.ascontiguousarray(b_qkv[2 * D:][col])[None, :],
            "wout": w_out,
            "bout": b_out[None, :],
            "gamma": ln_gamma[None, :].astype(np.float32),
            "beta": ln_beta[None, :].astype(np.float32),
            "cosM": cosM, "sinM": sinM, "SpermT": SpermT, "ident": ident,
            "xres": np.ascontiguousarray(xT[myb][:, 512 * myblk : 512 * myblk + 512]),
        }
        in_maps.append(m)
    return in_maps


def _assemble(results):
    out = np.zeros((B, S, D), dtype=np.float32)
    for c in range(NCORES):
        myb, myblk = c // 4, c % 4
        out[myb, 512 * myblk : 512 * myblk + 512, :] = results[c]["y_out"]
    return out


def run(trace=False, **inputs):
    """Full run returning (output, BassKernelResults) — used by test.py for
    profiling; kernel() below is the graded entry point."""
    in_maps = _prepare_in_maps(**inputs)
    res = bass_utils.run_bass_kernel_spmd(
        _get_nc(), in_maps, core_ids=list(range(NCORES)), trace=trace)
    return _assemble(res.results), res


def kernel(**inputs):
    out, _ = run(trace=False, **inputs)
    return out


# revision 23
# speedup vs baseline: 1.1583x; 1.0426x over previous
"""Multi-head rotary attention block on 8 Trainium2 NeuronCores.

Sharding: tensor-parallel over heads (16 heads / 8 cores = 2 heads per core,
both batches on every core); one 8-way AllToAll redistributes the attention
output from head-sharded to sequence-sharded for the output projection, so
each core finishes layernorm on its own [512, 1024] output slice.

Per-core dataflow (feature-major "T" layouts are [channels, seq]):
  qT/kT = w_q^T x^T (+bias via K=1 matmul) with RoPE applied as
          raw*cosM + (SpermT^T raw)*sinM (rotation permutation as a matmul)
  vT    = w_v^T x^T, PE-transposed to natural v [seq, 128]
  per head: sT[j,i] = kT^T qT (transposed scores), pT = exp(sT/32) on ScalarE
  (no max subtraction: scores are O(0.5) under this operator's input law),
  softmax denominators D via ones-matmul over pT, 1/D = exp(-ln(D)) on ScalarE,
  xvT = (v^T pT) * (1/D); AllToAll; z = xv_gathered + x residual;
  y = z^T w_out + b_out; layernorm via bn_stats/bn_aggr + exp(-0.5 ln(var+eps)).

All matmuls run as float32r (full-rate fp32 PE mode; 4-byte data, producers
write into tiles declared float32r as the BIR verifier requires).
"""
import numpy as np

import concourse.bass as bass
import concourse.bacc as bacc
import concourse.tile as tile
import concourse.mybir as mybir
from concourse import bass_utils

F32 = mybir.dt.float32
F32R = mybir.dt.float32r
AF = mybir.ActivationFunctionType
ALU = mybir.AluOpType

NCORES = 8
B, S, D = 2, 2048, 1024
HEADS, HD = 16, 64
SCALE = 1.0 / float(np.sqrt(D))  # reference scales by full D, not head_dim
IT = 512          # i-tile width for attention
N_IT = S // IT    # 4
JC = 128          # j-chunk
N_JC = S // JC    # 16
N_EC = D // 128   # 8 e-chunks


def _rope_consts():
    rot = HD // 2
    inv_freq = 1.0 / (10000.0 ** (np.arange(0, rot, 2, dtype=np.float64) / rot))
    ang = np.arange(S, dtype=np.float64)[:, None] * inv_freq[None, :]
    ang = np.repeat(ang, 2, axis=-1)  # [S, 32]
    cos, sin = np.cos(ang), np.sin(ang)
    cosM = np.ones((128, S), dtype=np.float32)
    sinM = np.zeros((128, S), dtype=np.float32)
    for base in (0, 64):
        cosM[base : base + 32, :] = cos.T.astype(np.float32)
        sinM[base : base + 32, :] = sin.T.astype(np.float32)
    Sp = np.zeros((128, 128), dtype=np.float32)
    for base in (0, 64):
        for m in range(32):
            r0 = base + m
            if m % 2 == 0:
                Sp[r0, r0 + 1] = -1.0
            else:
                Sp[r0, r0 - 1] = 1.0
    SpermT = np.ascontiguousarray(Sp.T)
    return cosM, sinM, SpermT


def _build(sim=False):
    nc = bacc.Bacc("TRN2", target_bir_lowering=False, debug=False, num_devices=NCORES)

    xT_d = [nc.dram_tensor(f"xT{b}", [D, S], F32R, kind="ExternalInput") for b in range(B)]
    wq_d = nc.dram_tensor("wq", [D, 128], F32R, kind="ExternalInput")
    wk_d = nc.dram_tensor("wk", [D, 128], F32R, kind="ExternalInput")
    wv_d = nc.dram_tensor("wv", [D, 128], F32R, kind="ExternalInput")
    bq_d = nc.dram_tensor("bq", [1, 128], F32R, kind="ExternalInput")
    bk_d = nc.dram_tensor("bk", [1, 128], F32R, kind="ExternalInput")
    bv_d = nc.dram_tensor("bv", [1, 128], F32R, kind="ExternalInput")
    wout_d = nc.dram_tensor("wout", [D, D], F32R, kind="ExternalInput")
    bout_d = nc.dram_tensor("bout", [1, D], F32R, kind="ExternalInput")
    gamma_d = nc.dram_tensor("gamma", [1, D], F32R, kind="ExternalInput")
    beta_d = nc.dram_tensor("beta", [1, D], F32R, kind="ExternalInput")
    cosM_d = nc.dram_tensor("cosM", [128, S], F32, kind="ExternalInput")
    sinM_d = nc.dram_tensor("sinM", [128, S], F32, kind="ExternalInput")
    spt_d = nc.dram_tensor("SpermT", [128, 128], F32R, kind="ExternalInput")
    ident_d = nc.dram_tensor("ident", [128, 128], F32, kind="ExternalInput")
    xres_d = nc.dram_tensor("xres", [D, 512], F32, kind="ExternalInput")
    y_d = nc.dram_tensor("y_out", [512, D], F32, kind="ExternalOutput")

    with tile.TileContext(nc) as tc:
        with (
            tc.tile_pool(name="persist", bufs=1) as pp,
            tc.tile_pool(name="dram", bufs=1, space="DRAM") as dram,
        ):
            cosM = pp.tile([128, S], F32, name="cosM_sb")
            sinM = pp.tile([128, S], F32, name="sinM_sb")
            spt = pp.tile([128, 128], F32R, name="spt_sb")
            ident = pp.tile([128, 128], F32, name="ident_sb")
            nc.sync.dma_start(cosM[:], cosM_d.ap())
            nc.sync.dma_start(sinM[:], sinM_d.ap())
            nc.sync.dma_start(spt[:], spt_d.ap())
            nc.sync.dma_start(ident[:], ident_d.ap())

            wq = pp.tile([128, N_EC, 128], F32R, name="wq_sb")  # [p, ec, m]
            wk = pp.tile([128, N_EC, 128], F32R, name="wk_sb")
            wv = pp.tile([128, N_EC, 128], F32R, name="wv_sb")
            for w_sb, w_dd in ((wq, wq_d), (wk, wk_d), (wv, wv_d)):
                nc.sync.dma_start(
                    w_sb[:], w_dd.ap().rearrange("(c p) m -> p c m", p=128))
            bq = pp.tile([1, 128], F32R, name="bq_sb")
            bk = pp.tile([1, 128], F32R, name="bk_sb")
            bv = pp.tile([1, 128], F32R, name="bv_sb")
            nc.sync.dma_start(bq[:], bq_d.ap())
            nc.sync.dma_start(bk[:], bk_d.ap())
            nc.sync.dma_start(bv[:], bv_d.ap())
            ones_row = pp.tile([1, IT], F32R, name="ones_row")
            nc.vector.memset(ones_row[:].bitcast(F32), 1.0)
            ones128 = pp.tile([128, 128], F32R, name="ones128")
            nc.vector.memset(ones128[:].bitcast(F32), 1.0)
            ones_col = pp.tile([1, 128], F32R, name="ones_col")
            nc.vector.memset(ones_col[:].bitcast(F32), 1.0)

            xvT = [pp.tile([128, S], F32, name=f"xvT_{b}") for b in range(B)]

            with (
                tc.tile_pool(name="psp", bufs=1, space="PSUM") as psp,
                tc.tile_pool(name="psa", bufs=1, space="PSUM") as psa,
                tc.tile_pool(name="ptmp", bufs=3) as ptmp,
                tc.tile_pool(name="ptp", bufs=8) as ptp,
                tc.tile_pool(name="rp", bufs=3) as rp,
            ):
             for b in range(B):
              with tc.tile_pool(name=f"qkv{b}", bufs=1) as qkvp:
                qTb = qkvp.tile([128, S], F32R, name=f"qT_{b}")
                kTb = qkvp.tile([128, S], F32R, name=f"kT_{b}")
                vnatb = [qkvp.tile([128, 130], F32R, name=f"v_{b}_{j}")
                         for j in range(N_JC)]
                for j in range(N_JC):
                    nc.vector.memset(vnatb[j][:, 64:65].bitcast(F32), 1.0)
                    nc.vector.memset(vnatb[j][:, 129:130].bitcast(F32), 1.0)
                # ---------- projections + rope for batch b ----------
                with (
                    tc.tile_pool(name=f"xt{b}", bufs=1) as xtp,
                ):
                    xt = xtp.tile([128, N_EC, S], F32R, name=f"xt_{b}")
                    xt_src = xT_d[b].ap().rearrange("(c p) s -> p c s", p=128)
                    for e in range(N_EC):
                        nc.sync.dma_start(xt[:, e, :], xt_src[:, e, :])

                    for w_sb, b_sb, dst in ((wq, bq, qTb), (wk, bk, kTb)):
                        for it in range(N_IT):
                            isl = slice(IT * it, IT * it + IT)
                            praw = psp.tile([128, IT], F32, tag="pqk")
                            for e in range(N_EC):
                                nc.tensor.matmul(
                                    praw[:], w_sb[:, e, :], xt[:, e, isl],
                                    start=(e == 0), stop=False)
                            nc.tensor.matmul(praw[:], b_sb[:], ones_row[:],
                                             start=False, stop=True)
                            raw = ptmp.tile([128, IT], F32R, tag="raw")
                            nc.vector.tensor_copy(raw[:], praw[:])
                            prot = psp.tile([128, IT], F32, tag="aux")
                            nc.tensor.matmul(prot[:], spt[:], raw[:],
                                             start=True, stop=True)
                            t1 = ptmp.tile([128, IT], F32, tag="t1")
                            nc.vector.tensor_tensor(t1[:], prot[:], sinM[:, isl], ALU.mult)
                            t2 = ptmp.tile([128, IT], F32, tag="t2")
                            nc.vector.tensor_tensor(
                                t2[:], raw[:].bitcast(F32), cosM[:, isl], ALU.mult)
                            nc.vector.tensor_tensor(dst[:, isl], t1[:], t2[:], ALU.add)

                    for it in range(N_IT):
                        isl = slice(IT * it, IT * it + IT)
                        pvt = psp.tile([128, IT], F32, tag="pqk")
                        for e in range(N_EC):
                            nc.tensor.matmul(pvt[:], wv[:, e, :], xt[:, e, isl],
                                             start=(e == 0), stop=False)
                        nc.tensor.matmul(pvt[:], bv[:], ones_row[:],
                                         start=False, stop=True)
                        vt_sb = ptmp.tile([128, IT], F32, tag="vt")
                        nc.vector.tensor_copy(vt_sb[:], pvt[:])
                        for jj in range(IT // 128):
                            jcc = it * (IT // 128) + jj
                            ptr_t = psp.tile([128, IT], F32, tag="aux", name="ptr_t")
                            ptr = ptr_t[:, 0:128]
                            nc.tensor.transpose(
                                ptr[:], vt_sb[:, 128 * jj : 128 * jj + 128], ident[:])
                            nc.vector.tensor_copy(vnatb[jcc][:, 0:64], ptr[:, 0:64])
                            nc.vector.tensor_copy(vnatb[jcc][:, 65:129], ptr[:, 64:128])

                # ---------- attention for batch b ----------
                if True:
                    for it in range(N_IT):
                        isl = slice(IT * it, IT * it + IT)
                        pxv = [psa.tile([128, IT], F32, tag="xv", bufs=2, name=f"pxv{_h}") for _h in range(2)]
                        for jc in range(N_JC):
                            jsl = slice(JC * jc, JC * jc + JC)
                            psc = [psa.tile([128, IT], F32, tag="sc", bufs=3, name=f"psc{_h}") for _h in range(2)]
                            pt = [ptp.tile([128, IT], F32R, tag="pt", name=f"pt{_h}") for _h in range(2)]
                            for hh in range(2):
                                hsl = slice(64 * hh, 64 * hh + 64)
                                nc.tensor.matmul(psc[hh][:], kTb[hsl, jsl],
                                                 qTb[hsl, isl], start=True, stop=True)
                                nc.scalar.activation(pt[hh][:], psc[hh][:], AF.Exp,
                                                     scale=SCALE)
                            first, last = jc == 0, jc == N_JC - 1
                            for hh in range(2):
                                nc.tensor.matmul(
                                    pxv[hh][0:65, :],
                                    vnatb[jc][:, 65 * hh : 65 * hh + 65],
                                    pt[hh][:], start=first, stop=last)
                        for hh in range(2):
                            rDf = rp.tile([128, IT], F32, tag="rDf")
                            nc.vector.reciprocal_approx_fast(
                                rDf[0:65, :], pxv[hh][0:65, :])
                            rD = rp.tile([128, IT], F32R, tag="rD")
                            nc.vector.tensor_copy(rD[0:1, :], rDf[64:65, :])
                            rDb = psa.tile([128, IT], F32, tag="rdb", bufs=1, name="rDb")
                            nc.tensor.matmul(rDb[0:64, :], ones_col[:, 0:64],
                                             rD[0:1, :], start=True, stop=True)
                            rDs = rp.tile([128, IT], F32, tag="rDs")
                            nc.vector.tensor_copy(rDs[0:64, :], rDb[0:64, :])
                            nc.vector.tensor_tensor(
                                xvT[b][64 * hh : 64 * hh + 64, isl],
                                pxv[hh][0:64, :], rDs[0:64, :], ALU.mult)

            # ---------- A2A ----------
            a2a_in = dram.tile([NCORES * 128, 512], F32)
            a2a_out = dram.tile([NCORES * 128, 512], F32)
            for j in range(NCORES):
                bj, blkj = j // 4, j % 4
                nc.sync.dma_start(
                    a2a_in[128 * j : 128 * j + 128, :],
                    xvT[bj][:, 512 * blkj : 512 * blkj + 512])
            if sim:
                # timing stand-in for TimelineSim (no collective support):
                # same-size DRAM->DRAM copy
                nc.sync.dma_start(a2a_out[:], a2a_in[:])
            else:
                nc.gpsimd.collective_compute(
                    "AllToAll", ALU.bypass,
                    replica_groups=[list(range(NCORES))],
                    ins=[a2a_in.opt()], outs=[a2a_out.opt()])

            # ---------- out-projection + layernorm ----------
            with (
                tc.tile_pool(name="wout_pool", bufs=1) as wp,
                tc.tile_pool(name="z_pool", bufs=1) as zp,
                tc.tile_pool(name="pso", bufs=2, space="PSUM") as pso,
                tc.tile_pool(name="ln_pool", bufs=2) as lnp,
            ):
                wout = wp.tile([128, N_EC, D], F32R, name="wout_sb")
                wout_src = wout_d.ap().rearrange("(c p) n -> p c n", p=128)
                for e in range(N_EC):
                    nc.sync.dma_start(wout[:, e, :], wout_src[:, e, :])
                bout = wp.tile([1, D], F32R, name="bout_sb")
                gamma = wp.tile([1, D], F32R, name="gamma_sb")
                beta = wp.tile([1, D], F32R, name="beta_sb")
                nc.sync.dma_start(bout[:], bout_d.ap())
                nc.sync.dma_start(gamma[:], gamma_d.ap())
                nc.sync.dma_start(beta[:], beta_d.ap())
                gbc = wp.tile([128, D], F32, name="gb_sb")
                bbc = wp.tile([128, D], F32, name="bb_sb")
                for half in range(2):
                    sl = slice(512 * half, 512 * half + 512)
                    pbc = pso.tile([128, 512], F32, tag="py", bufs=4)
                    nc.tensor.matmul(pbc[:], ones_col[:], gamma[:, sl],
                                     start=True, stop=True)
                    nc.scalar.copy(gbc[:, sl], pbc[:])
                    pbc2 = pso.tile([128, 512], F32, tag="py", bufs=4)
                    nc.tensor.matmul(pbc2[:], ones_col[:], beta[:, sl],
                                     start=True, stop=True)
                    nc.scalar.copy(bbc[:, sl], pbc2[:])

                eps_sb = zp.tile([128, 1], F32, name="eps_sb")
                nc.vector.memset(eps_sb[:], 1e-5)
                xres = [zp.tile([128, 512], F32, name=f"xres_{e}") for e in range(N_EC)]
                zT = [zp.tile([128, 512], F32R, name=f"zT_{e}") for e in range(N_EC)]
                for e in range(N_EC):
                    esl = slice(128 * e, 128 * e + 128)
                    nc.sync.dma_start(xres[e][:], xres_d.ap()[esl, :])
                    nc.sync.dma_start(zT[e][:], a2a_out[esl, :].bitcast(F32R))
                    nc.vector.tensor_tensor(
                        zT[e][:], zT[e][:].bitcast(F32), xres[e][:], ALU.add)

                for ic in range(4):
                    icl = slice(128 * ic, 128 * ic + 128)
                    py = [pso.tile([128, 512], F32, tag="py", bufs=4, name=f"py{_h}") for _h in range(2)]
                    for nh in range(2):
                        nsl = slice(512 * nh, 512 * nh + 512)
                        for e in range(N_EC):
                            nc.tensor.matmul(py[nh][:], zT[e][:, icl],
                                             wout[:, e, nsl],
                                             start=(e == 0), stop=False)
                        nc.tensor.matmul(py[nh][:], ones_col[:], bout[:, nsl],
                                         start=False, stop=True)
                    bn6 = lnp.tile([128, 2, 6], F32, tag="bn6")
                    nc.vector.bn_stats(bn6[:, 0, :], py[0][:])
                    nc.vector.bn_stats(bn6[:, 1, :], py[1][:])
                    bn2 = lnp.tile([128, 2], F32, tag="bn2")
                    nc.vector.bn_aggr(bn2[:], bn6[:])
                    lnv = lnp.tile([128, 1], F32, tag="lnv")
                    nc.scalar.activation(lnv[:], bn2[:, 1:2], AF.Ln, bias=eps_sb[:])
                    rstd = lnp.tile([128, 1], F32, tag="rstd")
                    nc.scalar.activation(rstd[:], lnv[:], AF.Exp, scale=-0.5)
                    yn = lnp.tile([128, D], F32, tag="yn")
                    for nh in range(2):
                        nsl = slice(512 * nh, 512 * nh + 512)
                        t = lnp.tile([128, 512], F32, tag="lt")
                        nc.vector.tensor_scalar(
                            t[:], py[nh][:], bn2[:, 0:1], rstd[:],
                            ALU.subtract, ALU.mult)
                        t2 = lnp.tile([128, 512], F32, tag="lt2")
                        nc.vector.tensor_tensor(t2[:], t[:], gbc[:, nsl], ALU.mult)
                        nc.vector.tensor_tensor(yn[:, nsl], t2[:], bbc[:, nsl], ALU.add)
                    nc.sync.dma_start(y_d.ap()[icl, :], yn[:])

    nc.compile()
    return nc


_NC_CACHE = None


def _get_nc():
    global _NC_CACHE
    if _NC_CACHE is None:
        _NC_CACHE = _build()
    return _NC_CACHE


def _prepare_in_maps(x, w_qkv, b_qkv, w_out, b_out, ln_gamma, ln_beta):
    x = np.asarray(x, dtype=np.float32)
    w_qkv = np.asarray(w_qkv, dtype=np.float32)
    b_qkv = np.asarray(b_qkv, dtype=np.float32)
    w_out = np.ascontiguousarray(np.asarray(w_out, dtype=np.float32))
    b_out = np.asarray(b_out, dtype=np.float32)
    ln_gamma = np.asarray(ln_gamma, dtype=np.float32)
    ln_beta = np.asarray(ln_beta, dtype=np.float32)

    cosM, sinM, SpermT = _rope_consts()
    ident = np.eye(128, dtype=np.float32)
    xT = [np.ascontiguousarray(x[b].T) for b in range(B)]

    in_maps = []
    for c in range(NCORES):
        h0 = 2 * c
        col = slice(HD * h0, HD * h0 + 128)
        myb, myblk = c // 4, c % 4
        m = {
            "xT0": xT[0], "xT1": xT[1],
            "wq": np.ascontiguousarray(w_qkv[:, col]),
            "wk": np.ascontiguousarray(w_qkv[:, D:][:, col]),
            "wv": np.ascontiguousarray(w_qkv[:, 2 * D:][:, col]),
            "bq": np.ascontiguousarray(b_qkv[col])[None, :],
            "bk": np.ascontiguousarray(b_qkv[D:][col])[None, :],
            "bv": np.ascontiguousarray(b_qkv[2 * D:][col])[None, :],
            "wout": w_out,
            "bout": b_out[None, :],
            "gamma": ln_gamma[None, :].astype(np.float32),
            "beta": ln_beta[None, :].astype(np.float32),
            "cosM": cosM, "sinM": sinM, "SpermT": SpermT, "ident": ident,
            "xres": np.ascontiguousarray(xT[myb][:, 512 * myblk : 512 * myblk + 512]),
        }
        in_maps.append(m)
    return in_maps


def _assemble(results):
    out = np.zeros((B, S, D), dtype=np.float32)
    for c in range(NCORES):
        myb, myblk = c // 4, c % 4
        out[myb, 512 * myblk : 512 * myblk + 512, :] = results[c]["y_out"]
    return out


def run(trace=False, **inputs):
    """Full run returning (output, BassKernelResults) — used by test.py for
    profiling; kernel() below is the graded entry point."""
    in_maps = _prepare_in_maps(**inputs)
    res = bass_utils.run_bass_kernel_spmd(
        _get_nc(), in_maps, core_ids=list(range(NCORES)), trace=trace)
    return _assemble(res.results), res


def kernel(**inputs):
    out, _ = run(trace=False, **inputs)
    return out
